# revision 25
# baseline (speedup 1.0000x reference)
"""ChirpletKANLinear forward on 8 Trainium2 NeuronCores.

Math (per reference):
    base_out[b,o]  = sum_i silu(x[b,i]) * BW[o,i]
    xs             = (x[b,i] - T[o,i]) / S[o,i]
    chirp[b,o,i]   = cos(2*pi*F[o,i]*xs) * exp(-0.5*xs^2)
    out[b,o]       = base_out + sum_i chirp * CW[o,i] + bias[o]

Sharding: out-features across the 8 cores (64 each), full batch per core.

Per (o, i-chunk) tile of [128 i, 1024 b], with tiles processed in groups and
quad-batched where the op's scalars are constant (4x fewer instructions):
    DVE: mf = int32(65536*(u2*x + v2))   per-tile  (u2 = F/S, v2 = 1/4 - F*T/S)
    DVE: fr = (mf << 16) >> 16           per-tile  (frac of phase in turns,
                                         wrapped to [-0.5, 0.5) by sign ext)
    ACT: sinv = Sin(fr * 2pi/65536)      quad-batched = cos(2*pi*F*xs)
    ACT: derf = Derivative_Erf(w*x + p)  per-tile  = 2/sqrt(pi) exp(-0.5 xs^2)
    DVE: g = sinv * derf                 quad-batched (bf16)
    PE : psum[32-strip, b] += lhsT^T @ g with lhsT = sparse column
         sqrt(pi)/2 * CW (M=32 col-tiling, tile_position by o-strip)
Tiles run in phases of G: a Sin phase (one table set) buffers G/QW cos
quads, then a Derivative_Erf phase (other table set) produces the gaussian
quads and immediately multiplies + matmuls, so ACT table loads amortize
over G tiles. The per-tile phase affines (mf) for the next group are
emitted inside the current derf phase, where the DVE has slack, so the
ACT sin phase never starves.
"""

import math

import numpy as np
import ml_dtypes

import concourse.bass as bass
import concourse.bacc as bacc
import concourse.tile as tile
import concourse.mybir as mybir
from concourse.bass_utils import run_bass_kernel_spmd

B, IN, OUT = 1024, 512, 512
NCORES = 8
OSH = OUT // NCORES          # 64 out features per core
NCH = IN // 128              # 4 contraction chunks of 128 partitions
QW = 4                       # quad width for batched constant-scalar ops
G = 32                       # tiles per ACT table-set phase (8 quads)
HALF = B // 2                # 512 fp32 = one PSUM bank per matmul

F32 = mybir.dt.float32
I32 = mybir.dt.int32
BF16 = mybir.dt.bfloat16
AF = mybir.ActivationFunctionType
ALU = mybir.AluOpType
TWO_PI = 2.0 * math.pi

TRACE = False
LAST_RESULT = None

_nc_cache = None


def _build_nc(loop_r=None):
    nc = bacc.Bacc("TRN2", target_bir_lowering=False, debug=False,
                   num_devices=NCORES)

    xT_d = nc.dram_tensor("xT", [NCH, 128, B], F32, kind="ExternalInput")
    # [p, c, j, o]: j = 0 sin-scale, 1 sin-bias, 2 gauss-scale, 3 gauss-bias,
    # 4 base-weight lhsT column
    pf32_d = nc.dram_tensor("pf32", [128, NCH, 5, OSH], F32,
                            kind="ExternalInput")
    # sparse lhsT columns for the 32-wide output strips
    cwsp_d = nc.dram_tensor("cwsp", [128, NCH, OSH, 32], BF16,
                            kind="ExternalInput")
    bias_d = nc.dram_tensor("biasv", [OSH, 1], F32, kind="ExternalInput")
    out_d = nc.dram_tensor("out", [OSH, B], F32, kind="ExternalOutput")

    with tile.TileContext(nc) as tc:
        with (
            tc.tile_pool(name="singles", bufs=1) as singles,
            tc.tile_pool(name="mfpool", bufs=2) as mfpool,
            tc.tile_pool(name="dpool", bufs=2) as dpool,
            tc.tile_pool(name="quadpool", bufs=10) as quadpool,
            tc.tile_pool(name="gpool", bufs=2) as gpool,
            tc.tile_pool(name="psum", bufs=1,
                         space=bass.MemorySpace.PSUM) as psump,
        ):
            xT_sb = singles.tile([128, NCH, B], F32)
            for c in range(NCH):
                nc.sync.dma_start(xT_sb[:, c, :], xT_d[c])
            pf32_sb = singles.tile([128, NCH, 5, OSH], F32)
            nc.sync.dma_start(pf32_sb[:], pf32_d[:])
            cwsp_sb = singles.tile([128, NCH, OSH, 32], BF16)
            nc.sync.dma_start(cwsp_sb[:], cwsp_d[:])
            bias_sb = singles.tile([OSH, 1], F32)
            nc.sync.dma_start(bias_sb[:], bias_d[:])

            psum_acc = psump.tile([OSH, B], F32)

            def compute_body():
                # Silu first: same ACT table set as Sin (silu_and_others).
                # Two half-size tiles share quadpool slots (bf16 q4 slots are
                # 8KB/partition; [128, 2, B] f32 is also 8KB) and recycle
                # after the base matmuls consume them.
                silu_ab = [quadpool.tile([128, 2, B], F32, tag="q4",
                                         name=f"silu_{i}")
                           for i in range(2)]
                for c in range(NCH):
                    nc.scalar.activation(silu_ab[c // 2][:, c % 2, :],
                                         xT_sb[:, c, :], AF.Silu)

                # Base-path matmuls open the PSUM accumulation groups.
                for h in range(2):
                    for c in range(NCH):
                        nc.tensor.matmul(
                            psum_acc[:, h * HALF:(h + 1) * HALF],
                            pf32_sb[:, c, 4, :],
                            silu_ab[c // 2][:, c % 2,
                                            h * HALF:(h + 1) * HALF],
                            start=(c == 0), stop=False,
                            skip_group_check=True,
                        )

                tiles_l = [(o, c) for c in range(NCH) for o in range(OSH)]
                ntiles = len(tiles_l)
                quads = [tiles_l[q:q + QW] for q in range(0, ntiles, QW)]
                qpg = G // QW      # quads per phase group

                def emit_m_quad(quad):
                    """Per-tile phase affines into one contiguous mf quad."""
                    mf4 = mfpool.tile([128, QW, B], I32, tag="mf4")
                    for qi, (o, c) in enumerate(quad):
                        nc.vector.tensor_scalar(
                            mf4[:, qi, :], xT_sb[:, c, :],
                            pf32_sb[:, c, 0, o:o + 1],
                            pf32_sb[:, c, 1, o:o + 1],
                            ALU.mult, ALU.add)
                    return mf4

                def emit_sin_quad(mf4):
                    """One batched shift + one batched Sin per quad."""
                    d4 = dpool.tile([128, QW, B], I32, tag="d4")
                    nc.vector.tensor_scalar(
                        d4[:], mf4[:], 16, 16,
                        ALU.arith_shift_left, ALU.arith_shift_right)
                    cos4 = quadpool.tile([128, QW, B], BF16, tag="q4")
                    nc.scalar.activation(cos4[:], d4[:], AF.Sin, bias=0.0,
                                         scale=TWO_PI / 65536.0)
                    return cos4

                def emit_derf_quad(quad):
                    ga4 = quadpool.tile([128, QW, B], BF16, tag="q4")
                    for qi, (o, c) in enumerate(quad):
                        nc.scalar.activation(
                            ga4[:, qi, :], xT_sb[:, c, :], AF.Derivative_Erf,
                            bias=pf32_sb[:, c, 3, o:o + 1],
                            scale=pf32_sb[:, c, 2, o:o + 1])
                    return ga4

                def emit_mult_mm(quad, cos4, ga4, qidx):
                    g4 = gpool.tile([128, QW, B], BF16, tag="g4")
                    nc.vector.tensor_tensor(g4[:], cos4[:], ga4[:], ALU.mult)
                    for qi, (o, c) in enumerate(quad):
                        strip = o // 32
                        last = (qidx * QW + qi == ntiles - 1)
                        for h in range(2):
                            nc.tensor.matmul(
                                psum_acc[:, h * HALF:(h + 1) * HALF]
                                [32 * strip:32 * strip + 32, :],
                                cwsp_sb[:, c, o, :],
                                g4[:, qi, h * HALF:(h + 1) * HALF],
                                start=False, stop=last,
                                skip_group_check=True,
                                tile_position=(0, 32 * strip),
                            )

                ngroups = len(quads) // qpg
                # m-quads for group 0 are emitted up front; thereafter each
                # derf phase interleaves the m-quads of the NEXT group so the
                # DVE does that work during its derf-phase slack.
                mf_bank = [emit_m_quad(q) for q in quads[0:2]]
                mf_next = 2
                for gi in range(ngroups):
                    gq = quads[gi * qpg:(gi + 1) * qpg]
                    cos_list = []
                    for k in range(qpg):
                        cos_list.append(emit_sin_quad(mf_bank.pop(0)))
                        # keep the mf bank ahead of the sin consumer
                        if mf_next < len(quads) and len(mf_bank) < 2:
                            mf_bank.append(emit_m_quad(quads[mf_next]))
                            mf_next += 1
                    for k, q in enumerate(gq):
                        ga4 = emit_derf_quad(q)
                        emit_mult_mm(q, cos_list[k], ga4, gi * qpg + k)
                        if mf_next < len(quads):
                            mf_bank.append(emit_m_quad(quads[mf_next]))
                            mf_next += 1

            if loop_r:
                with tc.For_i(0, loop_r, 1,
                              hint_engines=(mybir.EngineType.Activation,
                                            mybir.EngineType.DVE,
                                            mybir.EngineType.PE)):
                    compute_body()
            else:
                compute_body()

            out_sb = singles.tile([OSH, B], F32)
            nc.scalar.activation(out_sb, psum_acc, AF.Identity,
                                 bias=bias_sb[:, 0:1], scale=1.0)
            nc.sync.dma_start(out_d[:], out_sb[:])

    nc.compile()
    return nc


def _plane(a):
    """[OSH, IN] param -> [128 part, NCH, OSH] per-partition plane."""
    return np.ascontiguousarray(
        a.reshape(OSH, NCH, 128).transpose(2, 1, 0).astype(np.float32))


def _host_prep(inp):
    x = inp["x"]
    xT = np.ascontiguousarray(x.T.reshape(NCH, 128, B).astype(np.float32))
    maps = []
    for k in range(NCORES):
        sl = slice(k * OSH, (k + 1) * OSH)
        fk = inp["frequency"][sl]
        sk = inp["scale"][sl]
        tk = inp["translation"][sl]
        cwk = inp["chirplet_weights"][sl]
        bwk = inp["base_weight"][sl]
        u2 = (fk / sk) * 65536.0
        v2 = (0.25 - fk * tk / sk) * 65536.0
        w = 1.0 / (math.sqrt(2.0) * sk)
        p = -tk / (math.sqrt(2.0) * sk)
        lv = _plane((math.sqrt(math.pi) / 2.0) * cwk)    # [128, NCH, OSH]
        cwsp = np.zeros((128, NCH, OSH, 32), dtype=np.float32)
        cwsp[:, :, np.arange(OSH), np.arange(OSH) % 32] = lv
        pf32 = np.ascontiguousarray(np.stack(
            [_plane(u2), _plane(v2), _plane(w), _plane(p), _plane(bwk)],
            axis=2))                                     # [128, NCH, 5, OSH]
        maps.append({
            "xT": xT,
            "pf32": pf32,
            "cwsp": cwsp.astype(ml_dtypes.bfloat16),
            "biasv": np.ascontiguousarray(
                inp["bias"][sl].reshape(OSH, 1).astype(np.float32)),
        })
    return maps


def kernel(**inputs):
    global _nc_cache, LAST_RESULT
    np_in = {k: np.asarray(v, dtype=np.float32) for k, v in inputs.items()}
    if _nc_cache is None:
        _nc_cache = _build_nc()
    in_maps = _host_prep(np_in)
    res = run_bass_kernel_spmd(
        _nc_cache, in_maps, core_ids=list(range(NCORES)), trace=TRACE)
    LAST_RESULT = res
    shards = [r["out"] for r in res.results]          # each [OSH, B]
    full = np.concatenate(shards, axis=0)             # [OUT, B]
    return np.ascontiguousarray(full.T)               # [B, OUT] fp32
